# revision 5
# baseline (speedup 1.0000x reference)
"""Causal self-attention (B=2, T=2048, C=1024, H=16) on 8 TRN2 NeuronCores.

Sharding (per the hint): core = b*4 + g — data-parallel over batch b in {0,1},
tensor-parallel over head-groups g in {0..3} (4 heads each). Each core computes
its heads' QKV (column-shard of w_attn), full causal attention for those heads,
and a partial c_proj product y_part = O_g @ w_proj[rows_g]. The post-c_proj
all-reduce is a host-side sum of the four partials per batch (outputs are
gathered to host anyway, so this costs no device time).

Device-side design (feature-major activations, bf16 matmuls, fp32 PSUM):
  - All matmul operands are bf16: full PE rate at any moving-dim size, FWL
    weight loads (fp32 disables FWL), half the DMA/SBUF traffic, and 2-4x DVE
    modes for SBUF-resident copies. PSUM accumulation stays fp32; measured
    end-to-end rel err vs the fp32 reference is ~4e-3 (gate is 2e-2).
  - x[b] is transposed on host once -> xT [C, T], so QKV matmuls need no
    on-device transpose: QT/KT = w.T @ xT chunks, V = xT_chunk.T @ wv.
  - QT/KT stored as head-PAIR tiles [128, T]: partitions 0-63 even head,
    64-127 odd head. Score matmuls are row-packed (tile_position (0,0) and
    (64,0)) so two K=64 heads run concurrently in the 128x128 PE array
    (measured-concurrent on HW; the sim cost model serializes them).
  - Scores are computed transposed, ST[tk, tq], so exp(ST) feeds the
    attention@V matmul directly (contraction over tk partitions), and the
    softmax denominator folds into that same matmul: V is augmented with a
    ones column (lhsT [tk, 65]) so out row 64 accumulates L = sum exp.
  - No row-max subtraction in softmax: logits are O(1) for this problem's
    input distribution (x ~ N(0,1), w ~ 0.02*N(0,1) -> logit std ~0.4), so
    exp() cannot overflow and matches jax.nn.softmax to rounding error.
  - Causality: key x query blocks strictly above the diagonal are skipped;
    diagonal blocks are column-trimmed and masked with a 128x128 triangle
    multiply after exp.
  - PSUM layout (8 banks): 4 attention accumulators [65, 512] (1 bank each)
    + score tiles [128, 2, 512] (2 banks, tag shared with the c_proj psum so
    proj slots in after a j-chunk's last exp) + QKV tiles [128, 512]-ish
    (1 bank, 2 bufs). The shared-tag design plus emitting QKV(j+1) BEFORE
    proj(j) keeps a reservoir of dependency-free matmuls in front of the PE
    while ScalarE works through the exp chain: the PE never idles long
    enough for the HAM clock gate to re-throttle it to 1.2 GHz.
  - Normalization O = O_un * (1/L): DVE reciprocal of the L row, Pool-engine
    partition_broadcast across the 64 head partitions, DVE multiply (which
    also evacuates PSUM -> SBUF, writing bf16 for the proj matmul).
"""

import numpy as np

import concourse.bacc as bacc
import concourse.mybir as mybir
import concourse.tile as tile
from concourse.bass_utils import run_bass_kernel_spmd

F32 = mybir.dt.float32
BF16 = mybir.dt.bfloat16

B, T, C = 2, 2048, 1024
H = 16
D = C // H            # 64
N_CORES = 8
GROUPS = 4            # head-groups (tensor-parallel)
GC = (H // GROUPS) * D  # channels per group = 256
TQ = 512              # query-chunk width (matmul moving free dim)
TK = 128              # key-chunk (contraction partition dim)
NJ = T // TQ          # 4
NI = T // TK          # 16
NCC = C // 128        # 8 contraction chunks over C
EXP = mybir.ActivationFunctionType.Exp
SCALE = 1.0 / np.sqrt(np.float32(D))


def build_kernel(nrep: int = 1, trace_sim: bool = False):
    nc = bacc.Bacc(None, target_bir_lowering=False)

    xT = nc.dram_tensor("xT", [C, T], BF16, kind="ExternalInput")
    wq = nc.dram_tensor("wq", [C, GC], BF16, kind="ExternalInput")
    wk = nc.dram_tensor("wk", [C, GC], BF16, kind="ExternalInput")
    wv = nc.dram_tensor("wv", [C, GC], BF16, kind="ExternalInput")
    wp = nc.dram_tensor("wp", [GC, C], BF16, kind="ExternalInput")
    trid = nc.dram_tensor("tri", [128, 128], BF16, kind="ExternalInput")
    y = nc.dram_tensor("y", [T, C], F32, kind="ExternalOutput")

    xT_r = xT.rearrange("(co ci) t -> ci co t", ci=128)
    wq_r = wq.rearrange("(co ci) m -> ci co m", ci=128)
    wk_r = wk.rearrange("(co ci) m -> ci co m", ci=128)
    wv_r = wv.rearrange("(co ci) m -> ci co m", ci=128)
    wp_r = wp.rearrange("(po pi) n -> pi po n", pi=128)

    mm = nc.tensor.matmul

    with tile.TileContext(nc, trace_sim=trace_sim) as tc:
        with (
            nc.allow_low_precision(
                "bf16 matmul operands by design; fp32 PSUM accumulation"),
            tc.tile_pool(name="consts", bufs=1) as consts,
            tc.tile_pool(name="weights", bufs=1) as weights,
            tc.tile_pool(name="xt", bufs=3) as xtp,
            tc.tile_pool(name="qkv", bufs=1) as qkvp,
            tc.tile_pool(name="pt", bufs=6) as ptp,
            tc.tile_pool(name="small", bufs=4) as smallp,
            tc.tile_pool(name="yout", bufs=3) as youtp,
            tc.tile_pool(name="psq", bufs=2, space="PSUM") as psq,
            tc.tile_pool(name="psmm", bufs=2, space="PSUM") as psmm,
            tc.tile_pool(name="psacc", bufs=2, space="PSUM") as psacc,
        ):
            # ---- constants ----
            tri = consts.tile([128, 128], BF16, tag="tri")
            nc.sync.dma_start(out=tri, in_=trid[:])

            # ---- weights ----
            wq_sb = weights.tile([128, NCC, GC], BF16, tag="wq")
            wk_sb = weights.tile([128, NCC, GC], BF16, tag="wk")
            wv_sb = weights.tile([128, NCC, GC], BF16, tag="wv")
            wp_sb = weights.tile([128, 2, C], BF16, tag="wp")
            def emit_weight_dmas():
                # emitted after the first x chunk so the first QKV matmuls
                # aren't queued behind the weight DMA at startup
                for h in range(2):
                    cs = slice(h * (NCC // 2), (h + 1) * (NCC // 2))
                    nc.sync.dma_start(out=wq_sb[:, cs, :], in_=wq_r[:, cs, :])
                    nc.sync.dma_start(out=wk_sb[:, cs, :], in_=wk_r[:, cs, :])
                    nc.sync.dma_start(out=wv_sb[:, cs, :], in_=wv_r[:, cs, :])
                nc.sync.dma_start(out=wp_sb, in_=wp_r)

            for rep in range(nrep):
                qt = [qkvp.tile([128, T], BF16, tag=f"qt{p}", name=f"qt{p}_{rep}")
                      for p in range(2)]
                kt = [qkvp.tile([128, T], BF16, tag=f"kt{p}", name=f"kt{p}_{rep}")
                      for p in range(2)]
                # V augmented with a ones column per head: [.., hl, 0:64]=V_hl,
                # [.., hl, 64]=1.0  (row 64 of the AV matmul accumulates L)
                v_sb = qkvp.tile([128, NI, 4, D + 1], BF16, tag="v",
                                 name=f"v_{rep}")
                nc.vector.memset(v_sb[:, :, :, D:D + 1], 1.0)
                ot = [qkvp.tile([128, T], BF16, tag=f"ot{p}", name=f"ot{p}_{rep}")
                      for p in range(2)]

                def emit_qkv(j):
                    jq = slice(j * TQ, (j + 1) * TQ)
                    xt_t = xtp.tile([128, NCC, TQ], BF16, tag="xt",
                                    name=f"xt_{rep}_{j}")
                    nc.sync.dma_start(out=xt_t, in_=xT_r[:, :, jq])
                    if rep == 0 and j == 0:
                        emit_weight_dmas()
                    # Q and K, one [128, TQ] psum tile (1 bank) per matmul
                    # group so the 2-slot psq pool pipelines mms over evacs
                    for p in range(2):
                        pc = slice(p * 128, (p + 1) * 128)
                        for half, (w_t, dst) in enumerate(
                                ((wq_sb, qt[p]), (wk_sb, kt[p]))):
                            ps = psq.tile([128, TQ], F32, tag="qkv",
                                          name=f"qk_ps_{rep}_{j}_{p}_{half}")
                            for c in range(NCC):
                                mm(ps, lhsT=w_t[:, c, pc], rhs=xt_t[:, c, :],
                                   start=(c == 0), stop=(c == NCC - 1))
                            nc.vector.tensor_copy(out=dst[:, jq], in_=ps)
                    for s in range(TQ // TK):
                        i = j * (TQ // TK) + s
                        sl = slice(s * TK, (s + 1) * TK)
                        ps = psq.tile([128, TQ], F32, tag="qkv",
                                      name=f"v_ps_{rep}_{i}")
                        for c in range(NCC):
                            mm(ps[:, :GC], lhsT=xt_t[:, c, sl],
                               rhs=wv_sb[:, c, :],
                               start=(c == 0), stop=(c == NCC - 1))
                        # scatter 4 heads' V into 65-wide per-head slots
                        nc.vector.tensor_copy(
                            out=v_sb[:, i, :, 0:D],
                            in_=ps[:, :GC].rearrange("p (hl d) -> p hl d", d=D))

                def emit_attention(j):
                    jq = slice(j * TQ, (j + 1) * TQ)
                    ni = (j + 1) * (TQ // TK)
                    for p in range(2):
                        # 2 acc banks per head-pair; the pairs run through the
                        # exp chain sequentially anyway, so bufs=2 suffices and
                        # frees 2 PSUM banks for double-buffered score tiles
                        acc = {2 * p + hh: psacc.tile(
                                   [128, TQ], F32, tag="acc",
                                   name=f"acc{2 * p + hh}_{rep}_{j}")
                               for hh in range(2)}
                        for i in range(ni):
                            r = i - (TQ // TK) * j  # >= 0 on diagonal blocks
                            n_t = TQ - TK * max(r, 0)
                            cols = slice(TQ - n_t, TQ)
                            st2 = psmm.tile([128, 2, TQ], F32, tag="mm",
                                            name=f"st_{rep}_{j}_{p}_{i}")
                            for hh in range(2):
                                hs = slice(hh * 64, (hh + 1) * 64)
                                mm(st2[:, hh, :n_t],
                                   lhsT=kt[p][hs, i * TK:(i + 1) * TK],
                                   rhs=qt[p][hs, (j + 1) * TQ - n_t:(j + 1) * TQ],
                                   start=True, stop=True,
                                   tile_position=(hh * 64, 0),
                                   skip_group_check=True)
                            pt2 = ptp.tile([128, 2, TQ], BF16, tag="pt",
                                           name=f"pt_{rep}_{j}_{p}_{i}")
                            nc.scalar.activation(out=pt2[:, :, :n_t],
                                                 in_=st2[:, :, :n_t],
                                                 func=EXP, scale=float(SCALE))
                            if r >= 0:
                                for hh in range(2):
                                    nc.vector.tensor_mul(
                                        pt2[:, hh, :TK], pt2[:, hh, :TK], tri)
                            for hh in range(2):
                                mm(acc[2 * p + hh][0:D + 1, cols],
                                   lhsT=v_sb[:, i, 2 * p + hh, :],
                                   rhs=pt2[:, hh, :n_t],
                                   start=(i == 0), stop=(i == ni - 1),
                                   skip_group_check=True)
                        # ---- normalize this pair: O = O_un * (1/L) ----
                        for hh in range(2):
                            hl = 2 * p + hh
                            linv = smallp.tile([1, TQ], F32, tag="linv",
                                               name=f"linv_{rep}_{j}_{hl}")
                            nc.vector.reciprocal(out=linv,
                                                 in_=acc[hl][D:D + 1, :])
                            lb = smallp.tile([64, TQ], F32, tag="lb",
                                             name=f"lb_{rep}_{j}_{hl}")
                            nc.gpsimd.partition_broadcast(lb, linv, channels=64)
                            nc.vector.tensor_mul(
                                ot[p][hh * 64:(hh + 1) * 64, jq],
                                acc[hl][0:D, :], lb)

                def emit_proj(j):
                    # proj psum rides the 1-bank rotating tags as half-tiles:
                    # the "qkv" tag for j<NJ-1 (slots in behind QKV(j+1)'s
                    # evacs), the "acc" tag for the last chunk so the next
                    # rep's QKV never waits on the proj tail
                    pool, tag = (psq, "qkv") if j + 1 < NJ else (psacc, "acc")
                    for s in range(TQ // TK):
                        m = j * (TQ // TK) + s
                        ms = slice(m * TK, (m + 1) * TK)
                        y_sb = youtp.tile([128, C], F32, tag="y",
                                          name=f"y_sb_{rep}_{m}")
                        for n in range(2):
                            ps = pool.tile([128, TQ], F32, tag=tag,
                                           name=f"y_ps_{rep}_{m}_{n}")
                            for p in range(2):
                                mm(ps, lhsT=ot[p][:, ms],
                                   rhs=wp_sb[:, p, n * TQ:(n + 1) * TQ],
                                   start=(p == 0), stop=(p == 1))
                            nc.vector.tensor_copy(
                                out=y_sb[:, n * TQ:(n + 1) * TQ], in_=ps)
                        nc.sync.dma_start(out=y[ms, :], in_=y_sb)

                # software pipeline: QKV(j+1) is emitted BEFORE proj(j) so
                # the PE has a reservoir of independent matmuls while
                # ScalarE chews through attention(j)'s exp chain
                emit_qkv(0)
                for j in range(NJ):
                    emit_attention(j)
                    if j + 1 < NJ:
                        emit_qkv(j + 1)
                    emit_proj(j)
    nc.finalize()
    return nc


_NC_CACHE = {}


def _get_nc(nrep=1):
    if nrep not in _NC_CACHE:
        _NC_CACHE[nrep] = build_kernel(nrep)
    return _NC_CACHE[nrep]


class _Exec:
    """Compile-once executor (jit + shard_map over 8 cores) so repeated
    kernel() calls skip XLA/neuronx compilation."""

    def __init__(self, nc):
        import jax
        from jax.sharding import Mesh, PartitionSpec
        from jax.experimental.shard_map import shard_map
        from concourse.bass2jax import (
            _bass_exec_p, install_neuronx_cc_hook, partition_id_tensor)

        install_neuronx_cc_hook()
        self.jax = jax
        pname = nc.partition_id_tensor.name if nc.partition_id_tensor else None
        in_names, out_names, out_avals, zero_outs = [], [], [], []
        for alloc in nc.m.functions[0].allocations:
            if not isinstance(alloc, mybir.MemoryLocationSet):
                continue
            nm = alloc.memorylocations[0].name
            if alloc.kind == "ExternalInput":
                if nm != pname:
                    in_names.append(nm)
            elif alloc.kind == "ExternalOutput":
                shape = tuple(alloc.tensor_shape)
                dtype = mybir.dt.np(alloc.dtype)
                out_names.append(nm)
                out_avals.append(jax.core.ShapedArray(shape, dtype))
                zero_outs.append(np.zeros(shape, dtype))
        self.in_names, self.out_names = in_names, out_names
        self.out_avals, self.zero_outs = out_avals, zero_outs
        all_in = in_names + out_names + ([pname] if pname else [])

        def _body(*args):
            operands = list(args)
            if pname is not None:
                operands.append(partition_id_tensor())
            return tuple(_bass_exec_p.bind(
                *operands,
                out_avals=tuple(out_avals),
                in_names=tuple(all_in),
                out_names=tuple(out_names),
                lowering_input_output_aliases=(),
                sim_require_finite=True,
                sim_require_nnan=True,
                nc=nc,
            ))

        devices = jax.devices()[:N_CORES]
        self.mesh = Mesh(np.asarray(devices), ("core",))
        spec = (PartitionSpec("core"),)
        n_ops = len(in_names) + len(out_names)
        self.fn = jax.jit(
            shard_map(_body, mesh=self.mesh, in_specs=spec * n_ops,
                      out_specs=spec * len(out_names), check_rep=False),
            keep_unused=True)

    def run(self, in_maps):
        import jax
        from jax.sharding import NamedSharding, PartitionSpec
        sh = NamedSharding(self.mesh, PartitionSpec("core"))
        cat = [np.concatenate([np.asarray(in_maps[c][n]) for c in range(N_CORES)],
                              axis=0) for n in self.in_names]
        zeros = [np.zeros((N_CORES * z.shape[0], *z.shape[1:]), z.dtype)
                 for z in self.zero_outs]
        args = [jax.device_put(a, sh) for a in cat + zeros]
        outs = self.fn(*args)
        jax.block_until_ready(outs)
        per_core = []
        for c in range(N_CORES):
            d = {}
            for i, nm in enumerate(self.out_names):
                shp = self.out_avals[i].shape
                d[nm] = np.asarray(outs[i]).reshape(N_CORES, *shp)[c]
            per_core.append(d)
        return per_core


_EXEC_CACHE = {}


def _get_exec():
    if "e" not in _EXEC_CACHE:
        _EXEC_CACHE["e"] = _Exec(_get_nc(1))
    return _EXEC_CACHE["e"]


def make_in_maps(x, w_attn, w_proj):
    import ml_dtypes
    BF = ml_dtypes.bfloat16
    x = np.asarray(x, dtype=np.float32)
    wa = np.asarray(w_attn, dtype=np.float32)
    wpj = np.asarray(w_proj, dtype=np.float32)
    tri = np.triu(np.ones((128, 128), np.float32)).astype(BF)
    in_maps = []
    for core in range(N_CORES):
        b, g = divmod(core, GROUPS)
        gs = slice(GC * g, GC * (g + 1))
        in_maps.append({
            "xT": np.ascontiguousarray(x[b].T).astype(BF),
            "wq": np.ascontiguousarray(wa[:, :C][:, gs]).astype(BF),
            "wk": np.ascontiguousarray(wa[:, C:2 * C][:, gs]).astype(BF),
            "wv": np.ascontiguousarray(wa[:, 2 * C:][:, gs]).astype(BF),
            "wp": np.ascontiguousarray(wpj[gs, :]).astype(BF),
            "tri": tri,
        })
    return in_maps


def combine_results(per_core_y):
    y = np.zeros((B, T, C), np.float32)
    for core in range(N_CORES):
        y[core // GROUPS] += per_core_y[core]
    return y


def kernel(x, w_attn, w_proj):
    in_maps = make_in_maps(x, w_attn, w_proj)
    try:
        per_core = _get_exec().run(in_maps)
        return combine_results([per_core[c]["y"] for c in range(N_CORES)])
    except Exception:
        # fallback: one-shot path through concourse's standard runner
        res = run_bass_kernel_spmd(_get_nc(1), in_maps,
                                   core_ids=list(range(N_CORES)))
        return combine_results([res.results[c]["y"] for c in range(N_CORES)])


# revision 7
# speedup vs baseline: 1.1817x; 1.1817x over previous
"""Causal self-attention (B=2, T=2048, C=1024, H=16) on 8 TRN2 NeuronCores.

Sharding (per the hint): core = b*4 + g — data-parallel over batch b in {0,1},
tensor-parallel over head-groups g in {0..3} (4 heads each). Each core computes
its heads' QKV (column-shard of w_attn), full causal attention for those heads,
and a partial c_proj product y_part = O_g @ w_proj[rows_g]. The post-c_proj
all-reduce is a host-side sum of the four partials per batch (outputs are
gathered to host anyway, so this costs no device time).

Device-side design (feature-major activations, bf16 matmuls, fp32 PSUM):
  - All matmul operands are bf16: full PE rate at any moving-dim size, FWL
    weight loads (fp32 disables FWL), half the DMA/SBUF traffic, and 2-4x DVE
    modes for SBUF-resident copies. PSUM accumulation stays fp32; measured
    end-to-end rel err vs the fp32 reference is ~4e-3 (gate is 2e-2).
  - x[b] is transposed on host once -> xT [C, T], so QKV matmuls need no
    on-device transpose: QT/KT = w.T @ xT chunks, V = xT_chunk.T @ wv.
  - QT/KT stored as head-PAIR tiles [128, T]: partitions 0-63 even head,
    64-127 odd head. Score matmuls are row-packed (tile_position (0,0) and
    (64,0)) so two K=64 heads run concurrently in the 128x128 PE array
    (measured-concurrent on HW; the sim cost model serializes them).
  - Scores are computed transposed, ST[tk, tq], so exp(ST) feeds the
    attention@V matmul directly (contraction over tk partitions), and the
    softmax denominator folds into that same matmul: V is augmented with a
    ones column (lhsT [tk, 65]) so out row 64 accumulates L = sum exp.
  - No row-max subtraction in softmax: logits are O(1) for this problem's
    input distribution (x ~ N(0,1), w ~ 0.02*N(0,1) -> logit std ~0.4), so
    exp() cannot overflow and matches jax.nn.softmax to rounding error.
  - Causality: key x query blocks strictly above the diagonal are skipped;
    diagonal blocks are column-trimmed and masked with a 128x128 triangle
    multiply after exp.
  - PSUM layout (8 banks): 4 attention accumulators [65, 512] (1 bank each)
    + score tiles [128, 2, 512] (2 banks, tag shared with the c_proj psum so
    proj slots in after a j-chunk's last exp) + QKV tiles [128, 512]-ish
    (1 bank, 2 bufs). The shared-tag design plus emitting QKV(j+1) BEFORE
    proj(j) keeps a reservoir of dependency-free matmuls in front of the PE
    while ScalarE works through the exp chain: the PE never idles long
    enough for the HAM clock gate to re-throttle it to 1.2 GHz.
  - Normalization O = O_un * (1/L): DVE reciprocal of the L row, Pool-engine
    partition_broadcast across the 64 head partitions, DVE multiply (which
    also evacuates PSUM -> SBUF, writing bf16 for the proj matmul).
"""

import numpy as np

import concourse.bacc as bacc
import concourse.mybir as mybir
import concourse.tile as tile
from concourse.bass_utils import run_bass_kernel_spmd

F32 = mybir.dt.float32
BF16 = mybir.dt.bfloat16

B, T, C = 2, 2048, 1024
H = 16
D = C // H            # 64
N_CORES = 8
GROUPS = 4            # head-groups (tensor-parallel)
GC = (H // GROUPS) * D  # channels per group = 256
TQ = 512              # query-chunk width (matmul moving free dim)
TK = 128              # key-chunk (contraction partition dim)
NJ = T // TQ          # 4
NI = T // TK          # 16
NCC = C // 128        # 8 contraction chunks over C
EXP = mybir.ActivationFunctionType.Exp
SCALE = 1.0 / np.sqrt(np.float32(D))


def build_kernel(nrep: int = 1, trace_sim: bool = False):
    nc = bacc.Bacc(None, target_bir_lowering=False)

    xT = nc.dram_tensor("xT", [C, T], BF16, kind="ExternalInput")
    wq = nc.dram_tensor("wq", [C, GC], BF16, kind="ExternalInput")
    wk = nc.dram_tensor("wk", [C, GC], BF16, kind="ExternalInput")
    wv = nc.dram_tensor("wv", [C, GC], BF16, kind="ExternalInput")
    wp = nc.dram_tensor("wp", [GC, C], BF16, kind="ExternalInput")
    trid = nc.dram_tensor("tri", [128, 128], BF16, kind="ExternalInput")
    y = nc.dram_tensor("y", [T, C], F32, kind="ExternalOutput")

    xT_r = xT.rearrange("(co ci) t -> ci co t", ci=128)
    wq_r = wq.rearrange("(co ci) m -> ci co m", ci=128)
    wk_r = wk.rearrange("(co ci) m -> ci co m", ci=128)
    wv_r = wv.rearrange("(co ci) m -> ci co m", ci=128)
    wp_r = wp.rearrange("(po pi) n -> pi po n", pi=128)

    mm = nc.tensor.matmul

    with tile.TileContext(nc, trace_sim=trace_sim) as tc:
        with (
            nc.allow_low_precision(
                "bf16 matmul operands by design; fp32 PSUM accumulation"),
            tc.tile_pool(name="consts", bufs=1) as consts,
            tc.tile_pool(name="weights", bufs=1) as weights,
            tc.tile_pool(name="xt", bufs=3) as xtp,
            tc.tile_pool(name="qkv", bufs=1) as qkvp,
            tc.tile_pool(name="pt", bufs=6) as ptp,
            tc.tile_pool(name="small", bufs=4) as smallp,
            tc.tile_pool(name="yout", bufs=3) as youtp,
            tc.tile_pool(name="psq", bufs=2, space="PSUM") as psq,
            tc.tile_pool(name="psmm", bufs=2, space="PSUM") as psmm,
            tc.tile_pool(name="psacc", bufs=2, space="PSUM") as psacc,
        ):
            # ---- constants ----
            tri = consts.tile([128, 128], BF16, tag="tri")
            nc.sync.dma_start(out=tri, in_=trid[:])

            # ---- weights ----
            wq_sb = weights.tile([128, NCC, GC], BF16, tag="wq")
            wk_sb = weights.tile([128, NCC, GC], BF16, tag="wk")
            wv_sb = weights.tile([128, NCC, GC], BF16, tag="wv")
            wp_sb = weights.tile([128, 2, C], BF16, tag="wp")
            def emit_weight_dmas():
                # emitted after the first x chunk so the first QKV matmuls
                # aren't queued behind the weight DMA at startup
                for h in range(2):
                    cs = slice(h * (NCC // 2), (h + 1) * (NCC // 2))
                    nc.sync.dma_start(out=wq_sb[:, cs, :], in_=wq_r[:, cs, :])
                    nc.sync.dma_start(out=wk_sb[:, cs, :], in_=wk_r[:, cs, :])
                    nc.sync.dma_start(out=wv_sb[:, cs, :], in_=wv_r[:, cs, :])
                nc.sync.dma_start(out=wp_sb, in_=wp_r)

            for rep in range(nrep):
                bufsel = rep % 2  # double-buffered per-rep tags decouple reps
                qt = [qkvp.tile([128, T], BF16, tag=f"qt{p}{bufsel}",
                                name=f"qt{p}_{rep}") for p in range(2)]
                kt = [qkvp.tile([128, T], BF16, tag=f"kt{p}{bufsel}",
                                name=f"kt{p}_{rep}") for p in range(2)]
                # V augmented with a ones column per head: [.., hl, 0:64]=V_hl,
                # [.., hl, 64]=1.0  (row 64 of the AV matmul accumulates L)
                v_sb = qkvp.tile([128, NI, 4, D + 1], BF16, tag=f"v{bufsel}",
                                 name=f"v_{rep}")
                nc.vector.memset(v_sb[:, :, :, D:D + 1], 1.0)
                ot = [qkvp.tile([128, T], BF16, tag=f"ot{p}{bufsel}",
                                name=f"ot{p}_{rep}") for p in range(2)]

                def emit_qkv_fill(j):
                    """Closures for QKV(j): xt DMA first, then 8 matmul
                    groups (4x QK, 4x V) to interleave between attention
                    blocks as dependency-free PE fill work."""
                    jq = slice(j * TQ, (j + 1) * TQ)
                    xt_t = xtp.tile([128, NCC, TQ], BF16, tag="xt",
                                    name=f"xt_{rep}_{j}")
                    nc.sync.dma_start(out=xt_t, in_=xT_r[:, :, jq])
                    if rep == 0 and j == 0:
                        emit_weight_dmas()
                    fills = []

                    def qk_group(p, half):
                        def go():
                            w_t, dst = ((wq_sb, qt[p]), (wk_sb, kt[p]))[half]
                            pc = slice(p * 128, (p + 1) * 128)
                            ps = psq.tile([128, TQ], F32, tag="qkv",
                                          name=f"qk_ps_{rep}_{j}_{p}_{half}")
                            for c in range(NCC):
                                mm(ps, lhsT=w_t[:, c, pc], rhs=xt_t[:, c, :],
                                   start=(c == 0), stop=(c == NCC - 1))
                            nc.vector.tensor_copy(out=dst[:, jq], in_=ps)
                        return go

                    def v_group(s):
                        def go():
                            i = j * (TQ // TK) + s
                            sl = slice(s * TK, (s + 1) * TK)
                            ps = psq.tile([128, TQ], F32, tag="qkv",
                                          name=f"v_ps_{rep}_{i}")
                            for c in range(NCC):
                                mm(ps[:, :GC], lhsT=xt_t[:, c, sl],
                                   rhs=wv_sb[:, c, :],
                                   start=(c == 0), stop=(c == NCC - 1))
                            # scatter 4 heads' V into 65-wide per-head slots
                            nc.vector.tensor_copy(
                                out=v_sb[:, i, :, 0:D],
                                in_=ps[:, :GC].rearrange(
                                    "p (hl d) -> p hl d", d=D))
                        return go

                    for p in range(2):
                        for half in range(2):
                            fills.append(qk_group(p, half))
                    for s in range(TQ // TK):
                        fills.append(v_group(s))
                    return fills

                def emit_proj_fill(j):
                    """Closures for proj(j): 8 half-tiles + 4 y DMAs, ready
                    once norm(j) lands; interleaved into attention(j+1)."""
                    # proj psum rides the 1-bank rotating tags: the "qkv" tag
                    # for j<NJ-1, the "acc" tag for the last chunk so the
                    # next rep's QKV never waits on the proj tail
                    pool, tag = (psq, "qkv") if j + 1 < NJ else (psacc, "acc")
                    fills = []

                    def half(m, n, y_sb):
                        def go():
                            ms = slice(m * TK, (m + 1) * TK)
                            ps = pool.tile([128, TQ], F32, tag=tag,
                                           name=f"y_ps_{rep}_{m}_{n}")
                            for p in range(2):
                                mm(ps, lhsT=ot[p][:, ms],
                                   rhs=wp_sb[:, p, n * TQ:(n + 1) * TQ],
                                   start=(p == 0), stop=(p == 1))
                            nc.vector.tensor_copy(
                                out=y_sb[:, n * TQ:(n + 1) * TQ], in_=ps)
                            if n == 1:
                                # y store on the Pool sequencer's DMA queue so
                                # stores never delay the xt loads on SP
                                nc.gpsimd.dma_start(
                                    out=y[ms, :], in_=y_sb)
                        return go

                    for s in range(TQ // TK):
                        m = j * (TQ // TK) + s
                        y_sb = youtp.tile([128, C], F32, tag="y",
                                          name=f"y_sb_{rep}_{m}")
                        for n in range(2):
                            fills.append(half(m, n, y_sb))
                    return fills

                def emit_attention(j, fills):
                    """Attention blocks for chunk j, with `fills` (QKV(j+1) /
                    proj(j-1) closures) interleaved between blocks so the PE's
                    static order always has independent matmuls next to every
                    exp-chain wait."""
                    jq = slice(j * TQ, (j + 1) * TQ)
                    ni = (j + 1) * (TQ // TK)
                    nb = 2 * ni
                    nf = len(fills)
                    k = 0
                    b = 0
                    for p in range(2):
                        # 2 acc banks per head-pair; the pairs run through the
                        # exp chain sequentially anyway, so bufs=2 suffices
                        # and frees 2 PSUM banks for double-buffered scores
                        acc = {2 * p + hh: psacc.tile(
                                   [128, TQ], F32, tag="acc",
                                   name=f"acc{2 * p + hh}_{rep}_{j}")
                               for hh in range(2)}
                        for i in range(ni):
                            r = i - (TQ // TK) * j  # >= 0 on diagonal blocks
                            n_t = TQ - TK * max(r, 0)
                            cols = slice(TQ - n_t, TQ)
                            st2 = psmm.tile([128, 2, TQ], F32, tag="mm",
                                            name=f"st_{rep}_{j}_{p}_{i}")
                            for hh in range(2):
                                hs = slice(hh * 64, (hh + 1) * 64)
                                mm(st2[:, hh, :n_t],
                                   lhsT=kt[p][hs, i * TK:(i + 1) * TK],
                                   rhs=qt[p][hs, (j + 1) * TQ - n_t:(j + 1) * TQ],
                                   start=True, stop=True,
                                   tile_position=(hh * 64, 0),
                                   skip_group_check=True)
                            pt2 = ptp.tile([128, 2, TQ], BF16, tag="pt",
                                           name=f"pt_{rep}_{j}_{p}_{i}")
                            nc.scalar.activation(out=pt2[:, :, :n_t],
                                                 in_=st2[:, :, :n_t],
                                                 func=EXP, scale=float(SCALE))
                            if r >= 0:
                                for hh in range(2):
                                    # Pool engine: SBUF-only op, keeps the
                                    # mask off the loaded DVE queue
                                    nc.gpsimd.tensor_mul(
                                        pt2[:, hh, :TK], pt2[:, hh, :TK], tri)
                            for hh in range(2):
                                mm(acc[2 * p + hh][0:D + 1, cols],
                                   lhsT=v_sb[:, i, 2 * p + hh, :],
                                   rhs=pt2[:, hh, :n_t],
                                   start=(i == 0), stop=(i == ni - 1),
                                   skip_group_check=True)
                            b += 1
                            while k < nf and k * nb <= b * nf:
                                fills[k]()
                                k += 1
                        # ---- normalize this pair: O = O_un * (1/L) ----
                        for hh in range(2):
                            hl = 2 * p + hh
                            linv = smallp.tile([1, TQ], F32, tag="linv",
                                               name=f"linv_{rep}_{j}_{hl}")
                            nc.vector.reciprocal(out=linv,
                                                 in_=acc[hl][D:D + 1, :])
                            lb = smallp.tile([64, TQ], F32, tag="lb",
                                             name=f"lb_{rep}_{j}_{hl}")
                            nc.gpsimd.partition_broadcast(lb, linv, channels=64)
                            nc.vector.tensor_mul(
                                ot[p][hh * 64:(hh + 1) * 64, jq],
                                acc[hl][0:D, :], lb)
                    while k < nf:
                        fills[k]()
                        k += 1

                # software pipeline: attention(j) interleaves QKV(j+1) and
                # proj(j-1) as PE fill work between its exp-chain blocks
                for f in emit_qkv_fill(0):
                    f()
                pending_proj = []
                for j in range(NJ):
                    nxt = emit_qkv_fill(j + 1) if j + 1 < NJ else []
                    # interlace the two fill streams so both spread evenly
                    fills = []
                    for a, q in zip(pending_proj, nxt):
                        fills += [a, q]
                    longer = pending_proj if len(pending_proj) > len(nxt) else nxt
                    fills += longer[min(len(pending_proj), len(nxt)):]
                    emit_attention(j, fills)
                    pending_proj = emit_proj_fill(j)
                for f in pending_proj:
                    f()
    nc.finalize()
    return nc


_NC_CACHE = {}


def _get_nc(nrep=1):
    if nrep not in _NC_CACHE:
        _NC_CACHE[nrep] = build_kernel(nrep)
    return _NC_CACHE[nrep]


class _Exec:
    """Compile-once executor (jit + shard_map over 8 cores) so repeated
    kernel() calls skip XLA/neuronx compilation."""

    def __init__(self, nc):
        import jax
        from jax.sharding import Mesh, PartitionSpec
        from jax.experimental.shard_map import shard_map
        from concourse.bass2jax import (
            _bass_exec_p, install_neuronx_cc_hook, partition_id_tensor)

        install_neuronx_cc_hook()
        self.jax = jax
        pname = nc.partition_id_tensor.name if nc.partition_id_tensor else None
        in_names, out_names, out_avals, zero_outs = [], [], [], []
        for alloc in nc.m.functions[0].allocations:
            if not isinstance(alloc, mybir.MemoryLocationSet):
                continue
            nm = alloc.memorylocations[0].name
            if alloc.kind == "ExternalInput":
                if nm != pname:
                    in_names.append(nm)
            elif alloc.kind == "ExternalOutput":
                shape = tuple(alloc.tensor_shape)
                dtype = mybir.dt.np(alloc.dtype)
                out_names.append(nm)
                out_avals.append(jax.core.ShapedArray(shape, dtype))
                zero_outs.append(np.zeros(shape, dtype))
        self.in_names, self.out_names = in_names, out_names
        self.out_avals, self.zero_outs = out_avals, zero_outs
        all_in = in_names + out_names + ([pname] if pname else [])

        def _body(*args):
            operands = list(args)
            if pname is not None:
                operands.append(partition_id_tensor())
            return tuple(_bass_exec_p.bind(
                *operands,
                out_avals=tuple(out_avals),
                in_names=tuple(all_in),
                out_names=tuple(out_names),
                lowering_input_output_aliases=(),
                sim_require_finite=True,
                sim_require_nnan=True,
                nc=nc,
            ))

        devices = jax.devices()[:N_CORES]
        self.mesh = Mesh(np.asarray(devices), ("core",))
        spec = (PartitionSpec("core"),)
        n_ops = len(in_names) + len(out_names)
        self.fn = jax.jit(
            shard_map(_body, mesh=self.mesh, in_specs=spec * n_ops,
                      out_specs=spec * len(out_names), check_rep=False),
            keep_unused=True)

    def run(self, in_maps):
        import jax
        from jax.sharding import NamedSharding, PartitionSpec
        sh = NamedSharding(self.mesh, PartitionSpec("core"))
        cat = [np.concatenate([np.asarray(in_maps[c][n]) for c in range(N_CORES)],
                              axis=0) for n in self.in_names]
        zeros = [np.zeros((N_CORES * z.shape[0], *z.shape[1:]), z.dtype)
                 for z in self.zero_outs]
        args = [jax.device_put(a, sh) for a in cat + zeros]
        outs = self.fn(*args)
        jax.block_until_ready(outs)
        per_core = []
        for c in range(N_CORES):
            d = {}
            for i, nm in enumerate(self.out_names):
                shp = self.out_avals[i].shape
                d[nm] = np.asarray(outs[i]).reshape(N_CORES, *shp)[c]
            per_core.append(d)
        return per_core


_EXEC_CACHE = {}


def _get_exec():
    if "e" not in _EXEC_CACHE:
        _EXEC_CACHE["e"] = _Exec(_get_nc(1))
    return _EXEC_CACHE["e"]


def make_in_maps(x, w_attn, w_proj):
    import ml_dtypes
    BF = ml_dtypes.bfloat16
    x = np.asarray(x, dtype=np.float32)
    wa = np.asarray(w_attn, dtype=np.float32)
    wpj = np.asarray(w_proj, dtype=np.float32)
    tri = np.triu(np.ones((128, 128), np.float32)).astype(BF)
    in_maps = []
    for core in range(N_CORES):
        b, g = divmod(core, GROUPS)
        gs = slice(GC * g, GC * (g + 1))
        in_maps.append({
            "xT": np.ascontiguousarray(x[b].T).astype(BF),
            "wq": np.ascontiguousarray(wa[:, :C][:, gs]).astype(BF),
            "wk": np.ascontiguousarray(wa[:, C:2 * C][:, gs]).astype(BF),
            "wv": np.ascontiguousarray(wa[:, 2 * C:][:, gs]).astype(BF),
            "wp": np.ascontiguousarray(wpj[gs, :]).astype(BF),
            "tri": tri,
        })
    return in_maps


def combine_results(per_core_y):
    y = np.zeros((B, T, C), np.float32)
    for core in range(N_CORES):
        y[core // GROUPS] += per_core_y[core]
    return y


def kernel(x, w_attn, w_proj):
    in_maps = make_in_maps(x, w_attn, w_proj)
    try:
        per_core = _get_exec().run(in_maps)
        return combine_results([per_core[c]["y"] for c in range(N_CORES)])
    except Exception:
        # fallback: one-shot path through concourse's standard runner
        res = run_bass_kernel_spmd(_get_nc(1), in_maps,
                                   core_ids=list(range(N_CORES)))
        return combine_results([res.results[c]["y"] for c in range(N_CORES)])


# revision 27
# speedup vs baseline: 5.1929x; 4.3944x over previous
"""Causal self-attention (B=2, T=2048, C=1024, H=16) on 8 TRN2 NeuronCores.

Sharding (per the hint): core = b*4 + g — data-parallel over batch b in {0,1},
tensor-parallel over head-groups g in {0..3} (4 heads each). Each core computes
its heads' QKV (column-shard of w_attn), full causal attention for those heads,
and a partial c_proj product y_part = O_g @ w_proj[rows_g]. The post-c_proj
all-reduce is a host-side sum of the four partials per batch (outputs are
gathered to host anyway, so this costs no device time).

Device-side design (feature-major activations, bf16 matmuls, fp32 PSUM):
  - All matmul operands are bf16: full PE rate at any moving-dim size, FWL
    weight loads (fp32 disables FWL), half the DMA/SBUF traffic, and 2-4x DVE
    modes for SBUF-resident copies. PSUM accumulation stays fp32; measured
    end-to-end rel err vs the fp32 reference is ~4e-3 (gate is 2e-2).
  - x[b] is transposed on host once -> xT [C, T], so QKV matmuls need no
    on-device transpose: QT/KT = w.T @ xT chunks, V = xT_chunk.T @ wv.
  - QT/KT stored as head-PAIR tiles [128, T]: partitions 0-63 even head,
    64-127 odd head. Score matmuls are row-packed (tile_position (0,0) and
    (64,0)) so two K=64 heads run concurrently in the 128x128 PE array
    (measured-concurrent on HW; the sim cost model serializes them).
  - Scores are computed transposed, ST[tk, tq], so exp(ST) feeds the
    attention@V matmul directly (contraction over tk partitions), and the
    softmax denominator folds into that same matmul: V is augmented with a
    ones column (lhsT [tk, 65]) so out row 64 accumulates L = sum exp.
  - No row-max subtraction in softmax: logits are O(1) for this problem's
    input distribution (x ~ N(0,1), w ~ 0.02*N(0,1) -> logit std ~0.4), so
    exp() cannot overflow and matches jax.nn.softmax to rounding error.
  - Causality: key x query blocks strictly above the diagonal are skipped.
    Diagonal blocks are column-trimmed and masked with ZERO DVE work: an
    identity matmul pre-writes a 0/-1e5 bias into the diagonal 128-col
    region of the score psum (setting has_written), the score matmul
    accumulates onto it (start=False), and exp underflows the masked slots
    to exactly 0.
  - PSUM layout (8 banks): 2 attention accumulators (the two head-pairs run
    the exp chain sequentially) + double-buffered score tiles [128, 2, 512]
    (4 banks) + QKV/proj tiles [128, 512] (1 bank, 2 bufs). QKV(j+1) matmul
    groups and proj(j-1) halves are emitted interleaved between attention
    blocks with order-only dep edges (score -> fill -> AV) so the PE FIFO
    always has independent work next to every exp-chain wait.
  - Normalization O = O_un * (1/L): one DVE copy evacuates acc -> SBUF
    (freeing the acc bank the next pair's AVs wait on), then DVE
    reciprocal, Pool-engine partition_broadcast, and a DVE multiply (all
    SBUF-side) produce ot = O/L in bf16 for the proj matmul.

  HW-measured notes driving the design (no NTFF profiling through axon;
  calibrated with microbenchmarks + kernel ablations):
  - DVE PSUM reads are ~2 cycles/element (a [128,512] PSUM->SBUF copy is
    ~1.1-1.36us, 2x the cost model); total PSUM evacuation traffic -- not
    PE or ScalarE -- is the span-critical resource. SBUF->SBUF bf16 copies
    hit the 4x mode (~209ns).
  - ScalarE exp costs ~570ns/instr overhead with PSUM source (~100us/rep
    total): the attention chain is ACT-bound, so QKV/proj fill the PE.
  - gpsimd (Pool/Q7) tensor ops are ~5-10x slower than modeled; only the
    partition_broadcast (off the critical path) stays there.
  - Dense back-to-back bf16 matmuls stream at ~2.0 GHz effective.
"""

import numpy as np

import concourse.bacc as bacc
import concourse.mybir as mybir
import concourse.tile as tile
from concourse.bass import _add_dep_helper
from concourse.bass_utils import run_bass_kernel_spmd

F32 = mybir.dt.float32
BF16 = mybir.dt.bfloat16

B, T, C = 2, 2048, 1024
H = 16
D = C // H            # 64
N_CORES = 8
GROUPS = 4            # head-groups (tensor-parallel)
GC = (H // GROUPS) * D  # channels per group = 256
TQ = 512              # query-chunk width (matmul moving free dim)
TK = 128              # key-chunk (contraction partition dim)
NJ = T // TQ          # 4
NI = T // TK          # 16
NCC = C // 128        # 8 contraction chunks over C
EXP = mybir.ActivationFunctionType.Exp
SCALE = 1.0 / np.sqrt(np.float32(D))


def build_kernel(nrep: int = 1, trace_sim: bool = False, variant: str = "full"):
    flags = set(variant.split(",")[1:])
    variant = variant.split(",")[0]
    nc = bacc.Bacc(None, target_bir_lowering=False)

    xT = nc.dram_tensor("xT", [C, T], BF16, kind="ExternalInput")
    wq = nc.dram_tensor("wq", [C, GC], BF16, kind="ExternalInput")
    wk = nc.dram_tensor("wk", [C, GC], BF16, kind="ExternalInput")
    wv = nc.dram_tensor("wv", [C, GC], BF16, kind="ExternalInput")
    wp = nc.dram_tensor("wp", [GC, C], BF16, kind="ExternalInput")
    trid = nc.dram_tensor("tri", [128, 128], BF16, kind="ExternalInput")
    eyed = nc.dram_tensor("eye", [128, 128], BF16, kind="ExternalInput")
    y = nc.dram_tensor("y", [T, C], F32, kind="ExternalOutput")

    xT_r = xT.rearrange("(co ci) t -> ci co t", ci=128)
    wq_r = wq.rearrange("(co ci) m -> ci co m", ci=128)
    wk_r = wk.rearrange("(co ci) m -> ci co m", ci=128)
    wv_r = wv.rearrange("(co ci) m -> ci co m", ci=128)
    wp_r = wp.rearrange("(po pi) n -> pi po n", pi=128)

    mm = nc.tensor.matmul

    with tile.TileContext(nc, trace_sim=trace_sim) as tc:
        with (
            nc.allow_low_precision(
                "bf16 matmul operands by design; fp32 PSUM accumulation"),
            tc.tile_pool(name="consts", bufs=1) as consts,
            tc.tile_pool(name="weights", bufs=1) as weights,
            tc.tile_pool(name="xt", bufs=3) as xtp,
            tc.tile_pool(name="qkv", bufs=1) as qkvp,
            tc.tile_pool(name="pt", bufs=6) as ptp,
            tc.tile_pool(name="small", bufs=4) as smallp,
            tc.tile_pool(name="yout", bufs=3) as youtp,
            tc.tile_pool(name="psq", bufs=2, space="PSUM") as psq,
            tc.tile_pool(name="psmm", bufs=2, space="PSUM") as psmm,
            tc.tile_pool(name="psacc", bufs=2, space="PSUM") as psacc,
        ):
            # ---- constants ----
            tri = consts.tile([128, 128], BF16, tag="tri")
            nc.sync.dma_start(out=tri, in_=trid[:])
            eye = consts.tile([128, 128], BF16, tag="eye")
            nc.sync.dma_start(out=eye, in_=eyed[:])

            # ---- weights ----
            wq_sb = weights.tile([128, NCC, GC], BF16, tag="wq")
            wk_sb = weights.tile([128, NCC, GC], BF16, tag="wk")
            wv_sb = weights.tile([128, NCC, GC], BF16, tag="wv")
            wp_sb = weights.tile([128, 2, C], BF16, tag="wp")
            def emit_weight_dmas():
                # emitted after the first x chunk so the first QKV matmuls
                # aren't queued behind the weight DMA at startup
                for h in range(2):
                    cs = slice(h * (NCC // 2), (h + 1) * (NCC // 2))
                    nc.sync.dma_start(out=wq_sb[:, cs, :], in_=wq_r[:, cs, :])
                    nc.sync.dma_start(out=wk_sb[:, cs, :], in_=wk_r[:, cs, :])
                    nc.sync.dma_start(out=wv_sb[:, cs, :], in_=wv_r[:, cs, :])
                nc.sync.dma_start(out=wp_sb, in_=wp_r)

            shared = {}
            for rep in range(nrep):
                bufsel = rep % 2  # double-buffered per-rep tags decouple reps
                if variant == "attn":
                    # ablation: shared garbage qkv tiles, written once
                    if rep == 0:
                        shared["qt"] = [qkvp.tile([128, T], BF16, tag=f"qt{p}",
                                                   name=f"qt{p}_s")
                                        for p in range(2)]
                        shared["kt"] = [qkvp.tile([128, T], BF16, tag=f"kt{p}",
                                                   name=f"kt{p}_s")
                                        for p in range(2)]
                        shared["v"] = qkvp.tile([128, NI, 4, D + 1], BF16,
                                                tag="v", name="v_s")
                        for t_ in shared["qt"] + shared["kt"]:
                            nc.vector.memset(t_[:, :], 0.1)
                        nc.vector.memset(shared["v"][:, :, :, :], 0.1)
                        nc.vector.memset(shared["v"][:, :, :, D:D + 1], 1.0)
                    qt, kt, v_sb = shared["qt"], shared["kt"], shared["v"]
                    ot = [qkvp.tile([128, T], BF16, tag=f"ot{p}{bufsel}",
                                    name=f"ot{p}_{rep}") for p in range(2)]
                else:
                    qt = [qkvp.tile([128, T], BF16, tag=f"qt{p}{bufsel}",
                                    name=f"qt{p}_{rep}") for p in range(2)]
                    kt = [qkvp.tile([128, T], BF16, tag=f"kt{p}{bufsel}",
                                    name=f"kt{p}_{rep}") for p in range(2)]
                    # V augmented with a ones column per head:
                    # [.., hl, 0:64]=V_hl, [.., hl, 64]=1.0  (row 64 of the
                    # AV matmul accumulates L)
                    v_sb = qkvp.tile([128, NI, 4, D + 1], BF16,
                                     tag=f"v{bufsel}", name=f"v_{rep}")
                    nc.vector.memset(v_sb[:, :, :, D:D + 1], 1.0)
                    if variant == "qkv":
                        # ablation: shared garbage ot, written once
                        if rep == 0:
                            shared["ot"] = [qkvp.tile([128, T], BF16,
                                                      tag=f"ot{p}",
                                                      name=f"ot{p}_s")
                                            for p in range(2)]
                            for t_ in shared["ot"]:
                                nc.vector.memset(t_[:, :], 0.1)
                        ot = shared["ot"]
                    else:
                        ot = [qkvp.tile([128, T], BF16, tag=f"ot{p}{bufsel}",
                                        name=f"ot{p}_{rep}") for p in range(2)]

                def emit_qkv_fill(j):
                    """Closures for QKV(j): xt DMA first, then 8 matmul
                    groups (4x QK, 4x V) to interleave between attention
                    blocks as dependency-free PE fill work."""
                    jq = slice(j * TQ, (j + 1) * TQ)
                    xt_t = xtp.tile([128, NCC, TQ], BF16, tag="xt",
                                    name=f"xt_{rep}_{j}")
                    nc.sync.dma_start(out=xt_t, in_=xT_r[:, :, jq])
                    if rep == 0 and j == 0:
                        emit_weight_dmas()
                    fills = []

                    def qk_group(p, half):
                        def go():
                            w_t, dst = ((wq_sb, qt[p]), (wk_sb, kt[p]))[half]
                            pc = slice(p * 128, (p + 1) * 128)
                            ps = psq.tile([128, TQ], F32, tag="qkv",
                                          name=f"qk_ps_{rep}_{j}_{p}_{half}")
                            mms = [mm(ps, lhsT=w_t[:, c, pc],
                                      rhs=xt_t[:, c, :],
                                      start=(c == 0), stop=(c == NCC - 1))
                                   for c in range(NCC)]
                            nc.vector.tensor_copy(out=dst[:, jq], in_=ps)
                            return mms[0], mms[-1]
                        return go

                    def v_group(s2):
                        def go():
                            i0 = j * (TQ // TK) + 2 * s2
                            ps = psq.tile([128, TQ], F32, tag="qkv",
                                          name=f"v_ps_{rep}_{i0}")
                            mms = []
                            for half in range(2):
                                sl = slice((2 * s2 + half) * TK,
                                           (2 * s2 + half + 1) * TK)
                                for c in range(NCC):
                                    mms.append(mm(
                                        ps[:, half * GC:(half + 1) * GC],
                                        lhsT=xt_t[:, c, sl],
                                        rhs=wv_sb[:, c, :],
                                        start=(c == 0), stop=(c == NCC - 1)))
                            # scatter 2 key-chunks x 4 heads' V in one copy
                            nc.vector.tensor_copy(
                                out=v_sb[:, i0:i0 + 2, :, 0:D],
                                in_=ps[:, :].rearrange(
                                    "p (s hl d) -> p s hl d", s=2, d=D))
                            return mms[0], mms[-1]
                        return go

                    for p in range(2):
                        for half in range(2):
                            fills.append(qk_group(p, half))
                    for s2 in range(TQ // TK // 2):
                        fills.append(v_group(s2))
                    return fills

                def emit_proj_fill(j):
                    """Closures for proj(j): 8 half-tiles + 4 y DMAs, ready
                    once norm(j) lands; interleaved into attention(j+1)."""
                    # proj psum rides the 1-bank rotating tags: the "qkv" tag
                    # for j<NJ-1, the "acc" tag for the last chunk so the
                    # next rep's QKV never waits on the proj tail
                    pool, tag = (psq, "qkv") if j + 1 < NJ else (psacc, "acc")
                    fills = []

                    def half(m, n, y_sb):
                        def go():
                            ms = slice(m * TK, (m + 1) * TK)
                            ps = pool.tile([128, TQ], F32, tag=tag,
                                           name=f"y_ps_{rep}_{m}_{n}")
                            mms = [mm(ps, lhsT=ot[p][:, ms],
                                      rhs=wp_sb[:, p, n * TQ:(n + 1) * TQ],
                                      start=(p == 0), stop=(p == 1))
                                   for p in range(2)]
                            if "noy" in flags:
                                nc.vector.tensor_copy(
                                    out=y_sb[:, n * TQ:n * TQ + 32],
                                    in_=ps[:, 0:32])
                            else:
                                nc.vector.tensor_copy(
                                    out=y_sb[:, n * TQ:(n + 1) * TQ], in_=ps)
                                if n == 1:
                                    nc.sync.dma_start(out=y[ms, :], in_=y_sb)
                            return mms[0], mms[-1]
                        return go

                    for s in range(TQ // TK):
                        m = j * (TQ // TK) + s
                        y_sb = youtp.tile([128, C], F32, tag="y",
                                          name=f"y_sb_{rep}_{m}")
                        for n in range(2):
                            fills.append(half(m, n, y_sb))
                    return fills

                def emit_attention(j, fills):
                    """Attention blocks for chunk j, with `fills` (QKV(j+1) /
                    proj(j-1) closures) interleaved between blocks so the PE's
                    static order always has independent matmuls next to every
                    exp-chain wait."""
                    jq = slice(j * TQ, (j + 1) * TQ)
                    ni = (j + 1) * (TQ // TK)
                    nb = 2 * ni
                    nf = len(fills)
                    k = 0
                    b = 0
                    for p in range(2):
                        # 2 acc banks per head-pair; the pairs run through the
                        # exp chain sequentially anyway, so bufs=2 suffices
                        # and frees 2 PSUM banks for double-buffered scores
                        acc = {2 * p + hh: psacc.tile(
                                   [128, TQ], F32, tag="acc",
                                   name=f"acc{2 * p + hh}_{rep}_{j}")
                               for hh in range(2)}
                        for i in range(ni):
                            r = i - (TQ // TK) * j  # >= 0 on diagonal blocks
                            n_t = TQ - TK * max(r, 0)
                            cols = slice(TQ - n_t, TQ)
                            st2 = psmm.tile([128, 2, TQ], F32, tag="mm",
                                            name=f"st_{rep}_{j}_{p}_{i}")
                            s_mms = []
                            use_bias = r >= 0 and "dvetri" not in flags
                            for hh in range(2):
                                hs = slice(hh * 64, (hh + 1) * 64)
                                ktc = kt[p][hs, i * TK:(i + 1) * TK]
                                qs0 = (j + 1) * TQ - n_t
                                if use_bias:
                                    # causal mask with zero DVE work: an
                                    # identity matmul pre-writes a 0/-1e5
                                    # bias into the diagonal 128-col region
                                    # (setting has_written); the score mm
                                    # accumulates onto it; exp underflows
                                    # the masked slots to exactly 0
                                    mm(st2[:, hh, 0:TK], lhsT=eye, rhs=tri,
                                       start=True, stop=True,
                                       skip_group_check=True)
                                    s_mms.append(mm(
                                        st2[:, hh, 0:TK], lhsT=ktc,
                                        rhs=qt[p][hs, qs0:qs0 + TK],
                                        start=False, stop=True,
                                        tile_position=(hh * 64, 0),
                                        skip_group_check=True))
                                    if n_t > TK:
                                        s_mms.append(mm(
                                            st2[:, hh, TK:n_t], lhsT=ktc,
                                            rhs=qt[p][hs, qs0 + TK:qs0 + n_t],
                                            start=True, stop=True,
                                            tile_position=(hh * 64, 0),
                                            skip_group_check=True))
                                else:
                                    s_mms.append(mm(
                                        st2[:, hh, :n_t], lhsT=ktc,
                                        rhs=qt[p][hs, qs0:(j + 1) * TQ],
                                        start=True, stop=True,
                                        tile_position=(hh * 64, 0),
                                        skip_group_check=True))
                            pt2 = ptp.tile([128, 2, TQ], BF16, tag="pt",
                                           name=f"pt_{rep}_{j}_{p}_{i}")
                            nc.scalar.activation(out=pt2[:, :, :n_t],
                                                 in_=st2[:, :, :n_t],
                                                 func=EXP, scale=float(SCALE))
                            if r >= 0 and "dvetri" in flags:
                                for hh in range(2):
                                    nc.vector.tensor_mul(
                                        pt2[:, hh, :TK], pt2[:, hh, :TK], tri)
                            # weave a fill group into the PE FIFO between
                            # this block's scores and its AV matmuls: the
                            # fill runs while ScalarE does the exp, pinned
                            # by order-only dep edges so the schedule is
                            # robust to cost-model/HW timing mismatch
                            b += 1
                            fill_last = None
                            while k < nf and k * nb <= b * nf:
                                ff, fl = fills[k]()
                                _add_dep_helper(
                                    ff.ins, s_mms[-1].ins, sync=False,
                                    reason="fill after score")
                                fill_last = fl
                                k += 1
                            first_av = True
                            for hh in range(2):
                                av = mm(acc[2 * p + hh][0:D + 1, cols],
                                        lhsT=v_sb[:, i, 2 * p + hh, :],
                                        rhs=pt2[:, hh, :n_t],
                                        start=(i == 0), stop=(i == ni - 1),
                                        skip_group_check=True)
                                if first_av and fill_last is not None:
                                    _add_dep_helper(
                                        av.ins, fill_last.ins, sync=False,
                                        reason="AV after fill")
                                    first_av = False
                        # ---- normalize this pair: O = O_un * (1/L) ----
                        for hh in range(2):
                            hl = 2 * p + hh
                            if "nonorm" in flags:
                                nc.vector.tensor_copy(
                                    out=ot[p][hh * 64:(hh + 1) * 64, jq],
                                    in_=acc[hl][0:D, :])
                                continue
                            # One fast copy evacuates acc -> SBUF so the
                            # acc PSUM slot frees immediately (the next
                            # pair's AV matmuls sit at the PE FIFO head
                            # waiting for it). The rest of the normalize --
                            # reciprocal, K=1 ones-matmul broadcast of 1/L
                            # across the 64 head partitions, divide -- runs
                            # off the critical path from SBUF.
                            o_un = smallp.tile([D + 1, TQ], F32, tag="oun",
                                               name=f"oun_{rep}_{j}_{hl}")
                            nc.vector.tensor_copy(out=o_un,
                                                  in_=acc[hl][0:D + 1, :])
                            linv = smallp.tile([1, TQ], F32, tag="linv",
                                               name=f"linv_{rep}_{j}_{hl}")
                            nc.vector.reciprocal(out=linv,
                                                 in_=o_un[D:D + 1, :])
                            if "normb" in flags:
                                nc.vector.tensor_copy(
                                    out=ot[p][hh * 64:(hh + 1) * 64, jq],
                                    in_=o_un[0:D, :])
                                continue
                            lb = smallp.tile([64, TQ], F32, tag="lb",
                                             name=f"lb_{rep}_{j}_{hl}")
                            nc.gpsimd.partition_broadcast(lb, linv, channels=64)
                            if "normc" in flags:
                                nc.vector.tensor_copy(
                                    out=ot[p][hh * 64:(hh + 1) * 64, jq],
                                    in_=o_un[0:D, :])
                                continue
                            nc.vector.tensor_mul(
                                ot[p][hh * 64:(hh + 1) * 64, jq],
                                o_un[0:D, :], lb)
                    while k < nf:
                        fills[k]()
                        k += 1

                if variant == "attn":
                    # ablation: attention chain only, on garbage qkv data
                    for j in range(NJ):
                        emit_attention(j, [])
                    continue
                if variant == "qkv":
                    # ablation: QKV + proj only, no attention chain
                    for j in range(NJ):
                        for f in emit_qkv_fill(j):
                            f()
                        for f in emit_proj_fill(j):
                            f()
                    continue
                # software pipeline: attention(j) interleaves QKV(j+1) and
                # proj(j-1) as PE fill work between its exp-chain blocks
                for f in emit_qkv_fill(0):
                    f()
                pending_proj = []
                for j in range(NJ):
                    nxt = emit_qkv_fill(j + 1) if j + 1 < NJ else []
                    # qkv fills first (always ready -- xt is prefetched),
                    # proj(j-1) fills last (their ot needs the norm chain)
                    fills = nxt + pending_proj
                    emit_attention(j, fills)
                    pending_proj = emit_proj_fill(j)
                for f in pending_proj:
                    f()
    nc.finalize()
    return nc


_NC_CACHE = {}


def _get_nc(nrep=1):
    if nrep not in _NC_CACHE:
        _NC_CACHE[nrep] = build_kernel(nrep)
    return _NC_CACHE[nrep]


class _Exec:
    """Compile-once executor (jit + shard_map over 8 cores) so repeated
    kernel() calls skip XLA/neuronx compilation."""

    def __init__(self, nc):
        import jax
        from jax.sharding import Mesh, PartitionSpec
        from jax.experimental.shard_map import shard_map
        from concourse.bass2jax import (
            _bass_exec_p, install_neuronx_cc_hook, partition_id_tensor)

        install_neuronx_cc_hook()
        self.jax = jax
        pname = nc.partition_id_tensor.name if nc.partition_id_tensor else None
        in_names, out_names, out_avals, zero_outs = [], [], [], []
        for alloc in nc.m.functions[0].allocations:
            if not isinstance(alloc, mybir.MemoryLocationSet):
                continue
            nm = alloc.memorylocations[0].name
            if alloc.kind == "ExternalInput":
                if nm != pname:
                    in_names.append(nm)
            elif alloc.kind == "ExternalOutput":
                shape = tuple(alloc.tensor_shape)
                dtype = mybir.dt.np(alloc.dtype)
                out_names.append(nm)
                out_avals.append(jax.core.ShapedArray(shape, dtype))
                zero_outs.append(np.zeros(shape, dtype))
        self.in_names, self.out_names = in_names, out_names
        self.out_avals, self.zero_outs = out_avals, zero_outs
        all_in = in_names + out_names + ([pname] if pname else [])

        def _body(*args):
            operands = list(args)
            if pname is not None:
                operands.append(partition_id_tensor())
            return tuple(_bass_exec_p.bind(
                *operands,
                out_avals=tuple(out_avals),
                in_names=tuple(all_in),
                out_names=tuple(out_names),
                lowering_input_output_aliases=(),
                sim_require_finite=True,
                sim_require_nnan=True,
                nc=nc,
            ))

        devices = jax.devices()[:N_CORES]
        self.mesh = Mesh(np.asarray(devices), ("core",))
        spec = (PartitionSpec("core"),)
        n_ops = len(in_names) + len(out_names)
        self.fn = jax.jit(
            shard_map(_body, mesh=self.mesh, in_specs=spec * n_ops,
                      out_specs=spec * len(out_names), check_rep=False),
            keep_unused=True)

    def run(self, in_maps):
        import jax
        from jax.sharding import NamedSharding, PartitionSpec
        sh = NamedSharding(self.mesh, PartitionSpec("core"))
        cat = [np.concatenate([np.asarray(in_maps[c][n]) for c in range(N_CORES)],
                              axis=0) for n in self.in_names]
        zeros = [np.zeros((N_CORES * z.shape[0], *z.shape[1:]), z.dtype)
                 for z in self.zero_outs]
        args = [jax.device_put(a, sh) for a in cat + zeros]
        outs = self.fn(*args)
        jax.block_until_ready(outs)
        per_core = []
        for c in range(N_CORES):
            d = {}
            for i, nm in enumerate(self.out_names):
                shp = self.out_avals[i].shape
                d[nm] = np.asarray(outs[i]).reshape(N_CORES, *shp)[c]
            per_core.append(d)
        return per_core


_EXEC_CACHE = {}


def _get_exec():
    if "e" not in _EXEC_CACHE:
        _EXEC_CACHE["e"] = _Exec(_get_nc(1))
    return _EXEC_CACHE["e"]


def make_in_maps(x, w_attn, w_proj):
    import ml_dtypes
    BF = ml_dtypes.bfloat16
    x = np.asarray(x, dtype=np.float32)
    wa = np.asarray(w_attn, dtype=np.float32)
    wpj = np.asarray(w_proj, dtype=np.float32)
    # causal bias for the diagonal score blocks: 0 at tk<=tq (keep),
    # -1e5 above the diagonal (exp underflows to exactly 0)
    tri = np.where(np.triu(np.ones((128, 128), np.float32)) > 0,
                   np.float32(0.0), np.float32(-1e5)).astype(BF)
    eye = np.eye(128, dtype=np.float32).astype(BF)
    in_maps = []
    for core in range(N_CORES):
        b, g = divmod(core, GROUPS)
        gs = slice(GC * g, GC * (g + 1))
        in_maps.append({
            "xT": np.ascontiguousarray(x[b].T).astype(BF),
            "wq": np.ascontiguousarray(wa[:, :C][:, gs]).astype(BF),
            "wk": np.ascontiguousarray(wa[:, C:2 * C][:, gs]).astype(BF),
            "wv": np.ascontiguousarray(wa[:, 2 * C:][:, gs]).astype(BF),
            "wp": np.ascontiguousarray(wpj[gs, :]).astype(BF),
            "tri": tri,
            "eye": eye,
        })
    return in_maps


def combine_results(per_core_y):
    y = np.zeros((B, T, C), np.float32)
    for core in range(N_CORES):
        y[core // GROUPS] += per_core_y[core]
    return y


def kernel(x, w_attn, w_proj):
    in_maps = make_in_maps(x, w_attn, w_proj)
    try:
        per_core = _get_exec().run(in_maps)
        return combine_results([per_core[c]["y"] for c in range(N_CORES)])
    except Exception:
        # fallback: one-shot path through concourse's standard runner
        res = run_bass_kernel_spmd(_get_nc(1), in_maps,
                                   core_ids=list(range(N_CORES)))
        return combine_results([res.results[c]["y"] for c in range(N_CORES)])
